# revision 1
# baseline (speedup 1.0000x reference)
"""GQA + sliding-window attention Trainium2 kernel.

Problem: B=2, S=2048, EMB=2048, 16 Q heads / 4 KV heads, head=128,
causal sliding window of 1024 (inclusive), RoPE, output projection.

Sharding: 8 cores = 2 batches x 4 KV-head groups (4 Q heads per group).
Each core computes, for its (batch b, group g):
  q^T = (Wq_g x_b^T + bq), RoPE      (4 heads, transposed layout (hd, seq))
  k^T = (Wk_g x_b^T + bk), RoPE      (1 kv head)
  v   = x_b Wv_g^T + bv              (natural layout (seq, hd) via PE transpose)
  scores^T(k,q) = k^T.T-contracted   (hd contraction; (k_seq, q_seq) layout)
  exp (no max subtraction -- scores are O(1) here), window masks
  denom = ones^T @ exp               (column sums via PE)
  attn_out^T = v.T-contracted @ exp  (accumulate over k tiles)
  normalize by 1/denom (broadcast), then row-block of output projection:
  partial_out = attn^T.T @ Wo_g^T    (full (S, EMB), summed on host over g)
Host adds the 4 group partials per batch + bo.
"""

import math
import os

import numpy as np

S = 2048
EMB = 2048
HD = 128
QH = 4  # q heads per core (group)
NKV = 4  # kv heads total (= groups)
WINDOW = 1024
ROPE_THETA = 10000.0
SCALE = 1.0 / math.sqrt(HD)

_NC_CACHE = {}
LAST_RESULTS = None


def _build_nc():
    import concourse.mybir as mybir
    import concourse.tile as tile
    from concourse import bacc
    from concourse.masks import make_identity

    f32 = mybir.dt.float32
    f32r = mybir.dt.float32r
    AF = mybir.ActivationFunctionType

    nc = bacc.Bacc("TRN2", target_bir_lowering=False, debug=False)

    xT = nc.dram_tensor("xT", [EMB, S], f32r, kind="ExternalInput")
    wqT = nc.dram_tensor("wqT", [EMB, QH * HD], f32r, kind="ExternalInput")
    wkT = nc.dram_tensor("wkT", [EMB, HD], f32r, kind="ExternalInput")
    wvT = nc.dram_tensor("wvT", [EMB, HD], f32r, kind="ExternalInput")
    woT = nc.dram_tensor("woT", [QH * HD, EMB], f32r, kind="ExternalInput")
    bq_d = nc.dram_tensor("bq", [HD, QH], f32, kind="ExternalInput")
    bk_d = nc.dram_tensor("bk", [HD, 1], f32, kind="ExternalInput")
    bv_d = nc.dram_tensor("bv", [HD, 1], f32, kind="ExternalInput")
    cos_d = nc.dram_tensor("cosT", [HD, S], f32, kind="ExternalInput")
    sin_d = nc.dram_tensor("sinT", [HD, S], f32, kind="ExternalInput")
    m0_d = nc.dram_tensor("mask0", [128, 128], f32r, kind="ExternalInput")
    m8_d = nc.dram_tensor("mask8", [128, 128], f32r, kind="ExternalInput")
    out_d = nc.dram_tensor("out", [S, EMB], f32, kind="ExternalOutput")

    NE = EMB // 128  # contraction chunks
    NQT = S // 128  # 128-wide seq tiles
    QC = 256  # q chunk width in attention
    NC_CHUNK = S // QC

    def r(ap):
        return ap

    from contextlib import ExitStack

    with tile.TileContext(nc) as tc, ExitStack() as ctx_outer:
        with tc.tile_pool(name="const", bufs=1) as constp:
            ones_f = constp.tile([128, 1], f32)
            nc.vector.memset(ones_f, 1.0)
            ones_sb = constp.tile([128, 1], f32r)
            nc.vector.tensor_copy(ones_sb, ones_f)
            zero128 = constp.tile([128, 128], f32)
            nc.vector.memset(zero128, 0.0)
            ident = constp.tile([128, 128], f32)
            make_identity(nc, ident)
            m0 = constp.tile([128, 128], f32r)
            nc.sync.dma_start(m0, m0_d[:, :])
            m8 = constp.tile([128, 128], f32r)
            nc.sync.dma_start(m8, m8_d[:, :])
            bq_sb = constp.tile([HD, QH], f32)
            nc.sync.dma_start(bq_sb, bq_d[:, :])
            bk_sb = constp.tile([HD, 1], f32)
            nc.sync.dma_start(bk_sb, bk_d[:, :])
            bv_sb = constp.tile([HD, 1], f32)
            nc.sync.dma_start(bv_sb, bv_d[:, :])

            with tc.tile_pool(name="persist", bufs=1) as pers:
                q_sb = pers.tile([128, QH * S], f32r)
                k_sb = pers.tile([128, S], f32r)
                v_sb = pers.tile([128, S], f32r)
                attn_sb = pers.tile([128, QH * S], f32r)

                # ---- fused: projection + RoPE + attention, sliding over seq ----
                from concourse.dve_ops import (
                    RECIP_APPROX_FAST_CONSTS,
                    RECIPROCAL_APPROX_FAST,
                )

                mmp = ctx_outer.enter_context(
                    tc.tile_pool(name="mmpsum", bufs=2, space="PSUM")
                )
                vtp = ctx_outer.enter_context(
                    tc.tile_pool(name="vtpsum", bufs=1, space="PSUM")
                )
                sp = ctx_outer.enter_context(
                    tc.tile_pool(name="scpsum", bufs=2, space="PSUM")
                )
                avp = ctx_outer.enter_context(
                    tc.tile_pool(name="avpsum", bufs=2, space="PSUM")
                )
                dp = ctx_outer.enter_context(
                    tc.tile_pool(name="dnpsum", bufs=1, space="PSUM")
                )
                with (
                    tc.tile_pool(name="phaw", bufs=1) as wp,
                    tc.tile_pool(name="xin", bufs=2) as xp,
                    tc.tile_pool(name="ptmp", bufs=3) as tpool,
                    tc.tile_pool(name="expp", bufs=12) as ep,
                    tc.tile_pool(name="nrm", bufs=2) as nr,
                ):

                    wk_sb = wp.tile([128, NE * HD], f32r)
                    nc.sync.dma_start(
                        wk_sb.rearrange("p (a m) -> p a m", a=NE),
                        wkT.rearrange("(a p) m -> a p m", p=128).transpose([1, 0, 2]),
                    )
                    wv_sb = wp.tile([128, NE * HD], f32r)
                    nc.sync.dma_start(
                        wv_sb.rearrange("p (a m) -> p a m", a=NE),
                        wvT.rearrange("(a p) m -> a p m", p=128).transpose([1, 0, 2]),
                    )
                    cos_sb = wp.tile([HD, S], f32)
                    sin_sb = wp.tile([HD, S], f32)
                    # wq is loaded per contraction chunk, interleaved with the
                    # first x chunk, so projections start within ~2 us
                    wq_sb = wp.tile([128, NE * QH * HD], f32r)
                    wqT_v = wqT.rearrange("(a p) m -> a p m", p=128)

                    XC = QC  # seq chunk = attention q chunk (256)

                    def proj(xt, w_sb, wstride, col0, bias_ap):
                        ps = mmp.tile([128, 512], f32, tag="mm")
                        pss = ps[:, 0:XC]
                        for e in range(NE):
                            nc.tensor.matmul(
                                pss,
                                w_sb[:, e * wstride + col0 : e * wstride + col0 + HD],
                                xt[:, e * XC : (e + 1) * XC],
                                start=(e == 0),
                                stop=(e == NE - 1),
                            )
                        raw = tpool.tile([128, XC], f32, tag="praw")
                        nc.scalar.activation(raw, pss, AF.Identity, bias=bias_ap)
                        return raw

                    def rope(raw, sl, dst):
                        t1 = tpool.tile([128, XC], f32, tag="t1")
                        t2 = tpool.tile([128, XC], f32, tag="t2")
                        # rotate-half across partitions: DMA moves between
                        # partitions, then multiply/accumulate in place
                        nc.sync.dma_start(t2[0:64, :], raw[64:128, :])
                        nc.sync.dma_start(t2[64:128, :], raw[0:64, :])
                        nc.vector.tensor_mul(t1, raw, cos_sb[:, sl])
                        nc.vector.tensor_mul(t2, t2, sin_sb[:, sl])
                        nc.vector.tensor_add(dst, t1, t2)

                    for c in range(NC_CHUNK):
                        sl = slice(c * XC, (c + 1) * XC)
                        xt = xp.tile([128, NE * XC], f32r, tag="xt")
                        xT_v = xT[:, sl].rearrange("(a p) n -> a p n", p=128)
                        for e in range(NE):
                            nc.sync.dma_start(
                                xt[:, e * XC : (e + 1) * XC], xT_v[e]
                            )
                            if c == 0:
                                nc.sync.dma_start(
                                    wq_sb[:, e * QH * HD : (e + 1) * QH * HD],
                                    wqT_v[e],
                                )
                        if c == 0:
                            nc.sync.dma_start(cos_sb, cos_d[:, :])
                            nc.sync.dma_start(sin_sb, sin_d[:, :])
                        kraw = proj(xt, wk_sb, HD, 0, bk_sb[:, 0:1])
                        rope(kraw, sl, k_sb[:, sl])
                        vraw = proj(xt, wv_sb, HD, 0, bv_sb[:, 0:1])
                        for h in range(QH):
                            qraw = proj(xt, wq_sb, QH * HD, h * HD, bq_sb[:, h : h + 1])
                            rope(qraw, sl, q_sb[:, h * S + c * XC : h * S + (c + 1) * XC])
                        for j in range(XC // 128):
                            tps = vtp.tile([128, 128], f32, tag="vtr")
                            nc.tensor.transpose(
                                tps, vraw[:, j * 128 : (j + 1) * 128], ident
                            )
                            t0 = (c * XC) // 128 + j
                            nc.scalar.activation(
                                v_sb[:, t0 * 128 : (t0 + 1) * 128], tps, AF.Copy
                            )

                        # -------- attention for q-chunk c, all heads --------
                        kt_lo = max(0, 2 * c - 8)
                        kts = list(range(kt_lo, 2 * c + 2))
                        n = len(kts)
                        for h in range(QH):
                            qsl = slice(h * S + c * QC, h * S + (c + 1) * QC)
                            ets = []
                            for kt in kts:
                                ssp = sp.tile([128, QC], f32, tag="sc")
                                nc.tensor.matmul(
                                    ssp,
                                    k_sb[:, kt * 128 : (kt + 1) * 128],
                                    q_sb[:, qsl],
                                    start=True,
                                    stop=True,
                                )
                                et = ep.tile([128, QC], f32r, tag="et")
                                nc.scalar.activation(et, ssp, AF.Exp, scale=SCALE)
                                d0 = 2 * c - kt
                                d1 = d0 + 1
                                if d0 == -1:
                                    nc.vector.tensor_copy(et[:, 0:128], zero128)
                                elif d0 == 0:
                                    nc.vector.tensor_mul(et[:, 0:128], et[:, 0:128], m0)
                                elif d0 == 8:
                                    nc.vector.tensor_mul(et[:, 0:128], et[:, 0:128], m8)
                                if d1 == 0:
                                    nc.vector.tensor_mul(
                                        et[:, 128:256], et[:, 128:256], m0
                                    )
                                elif d1 == 8:
                                    nc.vector.tensor_mul(
                                        et[:, 128:256], et[:, 128:256], m8
                                    )
                                elif d1 == 9:
                                    nc.vector.tensor_copy(et[:, 128:256], zero128)
                                ets.append(et)
                            dn = dp.tile([1, QC], f32, tag="dn")
                            av = avp.tile([128, QC], f32, tag="av")
                            for i, et in enumerate(ets):
                                nc.tensor.matmul(
                                    dn, ones_sb, et, start=(i == 0), stop=(i == n - 1)
                                )
                            for i, et in enumerate(ets):
                                nc.tensor.matmul(
                                    av,
                                    v_sb[:, kts[i] * 128 : (kts[i] + 1) * 128],
                                    et,
                                    start=(i == 0),
                                    stop=(i == n - 1),
                                )
                            den_row = nr.tile([1, QC], f32, tag="dr")
                            nc.scalar.activation(den_row, dn, AF.Copy)
                            rec_row = nr.tile([1, QC], f32, tag="rr")
                            nc.vector._custom_dve(
                                RECIPROCAL_APPROX_FAST,
                                out=rec_row,
                                in0=den_row,
                                s0=RECIP_APPROX_FAST_CONSTS["s0"],
                                s1=RECIP_APPROX_FAST_CONSTS["s1"],
                                imm2=RECIP_APPROX_FAST_CONSTS["imm2"],
                            )
                            rec_b = nr.tile([128, QC], f32, tag="rb")
                            nc.gpsimd.partition_broadcast(rec_b, rec_row[0:1, :])
                            nc.vector.tensor_mul(attn_sb[:, qsl], av, rec_b)

                # ---------------- output projection ----------------
                with (
                    tc.tile_pool(name="wop", bufs=1) as wop,
                    tc.tile_pool(name="outp", bufs=3) as outp,
                ):
                    wo_sb = wop.tile([128, QH * EMB], f32r)
                    nc.sync.dma_start(
                        wo_sb.rearrange("p (a m) -> p a m", a=QH),
                        woT.rearrange("(a p) m -> a p m", p=128).transpose([1, 0, 2]),
                    )
                    OC = 512
                    for qt in range(NQT):
                        for ec in range(EMB // OC):
                            ops = mmp.tile([128, OC], f32, tag="mm")
                            for hh in range(QH):
                                nc.tensor.matmul(
                                    ops,
                                    attn_sb[
                                        :, hh * S + qt * 128 : hh * S + (qt + 1) * 128
                                    ],
                                    wo_sb[
                                        :, hh * EMB + ec * OC : hh * EMB + (ec + 1) * OC
                                    ],
                                    start=(hh == 0),
                                    stop=(hh == QH - 1),
                                )
                            ot = outp.tile([128, OC], f32, tag="ot")
                            nc.vector.tensor_copy(ot, ops)
                            nc.sync.dma_start(
                                out_d[
                                    qt * 128 : (qt + 1) * 128, ec * OC : (ec + 1) * OC
                                ],
                                ot,
                            )

    nc.compile()
    return nc


def _get_nc():
    if "nc" not in _NC_CACHE:
        _NC_CACHE["nc"] = _build_nc()
    return _NC_CACHE["nc"]


def _get_runner():
    """Build (once) a jitted 8-core shard_map runner for the bass module."""
    if "runner" in _NC_CACHE:
        return _NC_CACHE["runner"]

    import jax
    from jax.experimental.shard_map import shard_map
    from jax.sharding import Mesh, NamedSharding, PartitionSpec

    import concourse.mybir as mybir
    from concourse import bass2jax

    nc = _get_nc()
    bass2jax.install_neuronx_cc_hook()

    partition_name = (
        nc.partition_id_tensor.name if nc.partition_id_tensor else None
    )
    in_names, out_names, out_avals, zero_outs = [], [], [], []
    for alloc in nc.m.functions[0].allocations:
        if not isinstance(alloc, mybir.MemoryLocationSet):
            continue
        name = alloc.memorylocations[0].name
        if alloc.kind == "ExternalInput":
            if name != partition_name:
                in_names.append(name)
        elif alloc.kind == "ExternalOutput":
            shape = tuple(alloc.tensor_shape)
            dtype = mybir.dt.np(alloc.dtype)
            out_avals.append(jax.core.ShapedArray(shape, dtype))
            out_names.append(name)
            zero_outs.append(np.zeros(shape, dtype))
    n_params = len(in_names)
    all_names = in_names + out_names
    if partition_name is not None:
        all_names = all_names + [partition_name]

    def _body(*args):
        operands = list(args)
        if partition_name is not None:
            operands.append(bass2jax.partition_id_tensor())
        outs = bass2jax._bass_exec_p.bind(
            *operands,
            out_avals=tuple(out_avals),
            in_names=tuple(all_names),
            out_names=tuple(out_names),
            lowering_input_output_aliases=(),
            sim_require_finite=True,
            sim_require_nnan=True,
            nc=nc,
        )
        return tuple(outs)

    n_cores = 8
    devices = jax.devices()[:n_cores]
    mesh = Mesh(np.asarray(devices), ("core",))
    spec = PartitionSpec("core")
    sharded = jax.jit(
        shard_map(
            _body,
            mesh=mesh,
            in_specs=(spec,) * (n_params + len(out_names)),
            out_specs=(spec,) * len(out_names),
            check_rep=False,
        ),
        keep_unused=True,
    )
    sharding = NamedSharding(mesh, spec)
    runner = (sharded, in_names, out_names, out_avals, zero_outs, sharding)
    _NC_CACHE["runner"] = runner
    return runner


def _device_inputs(in_maps):
    """Concatenate per-core inputs along axis 0 and put them on device."""
    import jax

    sharded, in_names, out_names, out_avals, zero_outs, sharding = _get_runner()
    arrs = []
    for name in in_names:
        cat = np.concatenate([np.asarray(m[name]) for m in in_maps], axis=0)
        arrs.append(jax.device_put(cat, sharding))
    for z in zero_outs:
        cat = np.zeros((8 * z.shape[0], *z.shape[1:]), z.dtype)
        arrs.append(jax.device_put(cat, sharding))
    return arrs


def _run_on_device(dev_args):
    sharded, in_names, out_names, out_avals, zero_outs, sharding = _get_runner()
    out_arrs = sharded(*dev_args)
    results = []
    for c in range(8):
        results.append(
            {
                name: np.asarray(out_arrs[i]).reshape(8, *out_avals[i].shape)[c]
                for i, name in enumerate(out_names)
            }
        )
    return results


def _make_chained(n_iters):
    """jit of n_iters chained executions (outputs feed next call's output bufs).

    One dispatch round-trip, n_iters serial NEFF executions on device."""
    import jax
    from jax.experimental.shard_map import shard_map
    from jax.sharding import Mesh, PartitionSpec

    from concourse import bass2jax

    nc = _get_nc()
    sharded, in_names, out_names, out_avals, zero_outs, sharding = _get_runner()
    partition_name = nc.partition_id_tensor.name if nc.partition_id_tensor else None
    all_names = list(in_names) + list(out_names)
    if partition_name is not None:
        all_names = all_names + [partition_name]
    n_params = len(in_names)

    def _body_n(*args):
        ins = list(args[:n_params])
        outs = list(args[n_params:])
        for _ in range(n_iters):
            operands = ins + outs
            if partition_name is not None:
                operands.append(bass2jax.partition_id_tensor())
            outs = list(
                bass2jax._bass_exec_p.bind(
                    *operands,
                    out_avals=tuple(out_avals),
                    in_names=tuple(all_names),
                    out_names=tuple(out_names),
                    lowering_input_output_aliases=(),
                    sim_require_finite=True,
                    sim_require_nnan=True,
                    nc=nc,
                )
            )
        return tuple(outs)

    devices = jax.devices()[:8]
    mesh = Mesh(np.asarray(devices), ("core",))
    spec = PartitionSpec("core")
    n_out = len(out_names)
    return jax.jit(
        shard_map(
            _body_n,
            mesh=mesh,
            in_specs=(spec,) * (n_params + n_out),
            out_specs=(spec,) * n_out,
            check_rep=False,
        ),
        keep_unused=True,
    )


def bench_chained_ns(inputs, iters=24):
    """Device-serial exec time via chained executions in one dispatch."""
    import time

    import jax

    in_maps = _host_prep(
        np.asarray(inputs["x"], np.float32),
        np.asarray(inputs["Wq"], np.float32),
        np.asarray(inputs["bq"], np.float32),
        np.asarray(inputs["Wk"], np.float32),
        np.asarray(inputs["bk"], np.float32),
        np.asarray(inputs["Wv"], np.float32),
        np.asarray(inputs["bv"], np.float32),
        np.asarray(inputs["Wo"], np.float32),
        np.asarray(inputs["bo"], np.float32),
    )
    dev_args = _device_inputs(in_maps)
    f1 = _make_chained(1)
    fN = _make_chained(iters)
    jax.block_until_ready(f1(*dev_args))
    jax.block_until_ready(fN(*dev_args))
    reps = 3
    t1s, tNs = [], []
    for _ in range(reps):
        t0 = time.perf_counter()
        jax.block_until_ready(f1(*dev_args))
        t1s.append(time.perf_counter() - t0)
        t0 = time.perf_counter()
        jax.block_until_ready(fN(*dev_args))
        tNs.append(time.perf_counter() - t0)
    t1 = min(t1s)
    tN = min(tNs)
    return (tN - t1) / (iters - 1) * 1e9


def bench_ns(inputs, iters=20):
    """Average per-execution time (ns) over pipelined repeated runs."""
    import time

    import jax

    in_maps = _host_prep(
        np.asarray(inputs["x"], np.float32),
        np.asarray(inputs["Wq"], np.float32),
        np.asarray(inputs["bq"], np.float32),
        np.asarray(inputs["Wk"], np.float32),
        np.asarray(inputs["bk"], np.float32),
        np.asarray(inputs["Wv"], np.float32),
        np.asarray(inputs["bv"], np.float32),
        np.asarray(inputs["Wo"], np.float32),
        np.asarray(inputs["bo"], np.float32),
    )
    dev_args = _device_inputs(in_maps)
    sharded = _get_runner()[0]
    # warmup (compile + first exec)
    jax.block_until_ready(sharded(*dev_args))
    t0 = time.perf_counter()
    outs = None
    for _ in range(iters):
        outs = sharded(*dev_args)
    jax.block_until_ready(outs)
    t1 = time.perf_counter()
    return (t1 - t0) / iters * 1e9


def _host_prep(x, Wq, bq, Wk, bk, Wv, bv, Wo, bo):
    """Build the 8 per-core input maps."""
    pos = np.arange(S, dtype=np.float64)
    inv_freq = 1.0 / (ROPE_THETA ** (np.arange(0, HD, 2, dtype=np.float64) / HD))
    freqs = pos[None, :] * inv_freq[:, None]  # (64, S)
    cosT = np.empty((HD, S), np.float32)
    cosT[0:64] = np.cos(freqs)
    cosT[64:128] = np.cos(freqs)
    sinT = np.empty((HD, S), np.float32)
    sinT[0:64] = -np.sin(freqs)
    sinT[64:128] = np.sin(freqs)

    ii = np.arange(128)
    mask0 = (ii[:, None] <= ii[None, :]).astype(np.float32)  # k_off <= q_off
    mask8 = (ii[:, None] >= ii[None, :]).astype(np.float32)  # k_off >= q_off

    in_maps = []
    for core in range(8):
        b, g = core // NKV, core % NKV
        qs = slice(g * QH * HD, (g + 1) * QH * HD)
        ks = slice(g * HD, (g + 1) * HD)
        in_maps.append(
            {
                "xT": np.ascontiguousarray(x[b].T),
                "wqT": np.ascontiguousarray(Wq[qs].T),
                "wkT": np.ascontiguousarray(Wk[ks].T),
                "wvT": np.ascontiguousarray(Wv[ks].T),
                "woT": np.ascontiguousarray(Wo[:, qs].T),
                "bq": np.ascontiguousarray(bq[qs].reshape(QH, HD).T),
                "bk": np.ascontiguousarray(bk[ks].reshape(1, HD).T),
                "bv": np.ascontiguousarray(bv[ks].reshape(1, HD).T),
                "cosT": cosT,
                "sinT": sinT,
                "mask0": mask0,
                "mask8": mask8,
            }
        )
    return in_maps


def kernel(**inputs):
    x = np.asarray(inputs["x"], np.float32)
    bo = np.asarray(inputs["bo"], np.float32)
    in_maps = _host_prep(
        x,
        np.asarray(inputs["Wq"], np.float32),
        np.asarray(inputs["bq"], np.float32),
        np.asarray(inputs["Wk"], np.float32),
        np.asarray(inputs["bk"], np.float32),
        np.asarray(inputs["Wv"], np.float32),
        np.asarray(inputs["bv"], np.float32),
        np.asarray(inputs["Wo"], np.float32),
        bo,
    )
    results = _run_on_device(_device_inputs(in_maps))

    out = np.empty((2, S, EMB), np.float32)
    for b in range(2):
        acc = results[b * NKV]["out"].astype(np.float32).copy()
        for g in range(1, NKV):
            acc += results[b * NKV + g]["out"]
        out[b] = acc + bo[None, :]
    return out



# revision 2
# speedup vs baseline: 11.6109x; 11.6109x over previous
"""GQA + sliding-window attention Trainium2 kernel.

Problem: B=2, S=2048, EMB=2048, 16 Q heads / 4 KV heads, head=128,
causal sliding window of 1024 (inclusive), RoPE, output projection.
Sharding: 8 cores = 2 batches x 4 KV-head groups (4 Q heads per group).

v5 design:
  - bf16 matmul operands; fp32 PSUM accumulation; N=512 moving operands.
  - head-PAIR merged attention ([128,512] tiles cover 2 heads).
  - V computed directly in [seq, hd] layout (no PE transpose, no ident).
  - biases folded into the matmul chains as K=1 outer-product matmuls.
  - denominator on PE (ones^T @ et), reciprocal on [1,512], Pool
    partition_broadcast.
  - fully-masked half-tiles skipped in scores + exp (strided rhs).
  - out-proj interleaved per chunk; stores on SWDGE; rope DMAs on the
    ACT HWDGE queue; x/weight loads on the sync HWDGE queue.
"""

import math

import numpy as np

S = 2048
EMB = 2048
HD = 128
QH = 4
NKV = 4
WINDOW = 1024
ROPE_THETA = 10000.0
SCALE = 1.0 / math.sqrt(HD)

XC = 512
NCH = S // XC
QC = 256
NE = EMB // 128

_NC_CACHE = {}

# psum pool sizing knobs
SPP_BUFS = 3
AVP_BUFS = 2
MMP_BUFS = 2


def _emit_body(nc, tc, d, rep):
    import concourse.mybir as mybir

    f32 = mybir.dt.float32
    bf16 = mybir.dt.bfloat16
    AF = mybir.ActivationFunctionType

    (xT, wqT, wkT, wvT, woT, bq_d, bk_d, bv_d, cos_d, sin_d, m0_d, m8_d, out_d) = d

    from contextlib import ExitStack

    with ExitStack() as ctx:
        from concourse.masks import make_identity

        constp = ctx.enter_context(tc.tile_pool(name=f"const{rep}", bufs=1))
        identf = constp.tile([128, 128], f32)
        make_identity(nc, identf)
        ident = constp.tile([128, 128], bf16)
        nc.vector.tensor_copy(ident, identf)
        ones_f = constp.tile([128, 1], f32)
        nc.vector.memset(ones_f, 1.0)
        ones_bf = constp.tile([128, 1], bf16)
        nc.vector.tensor_copy(ones_bf, ones_f)
        onesrow_f = constp.tile([1, XC], f32)
        nc.vector.memset(onesrow_f, 1.0)
        onesrow = constp.tile([1, XC], bf16)
        nc.vector.tensor_copy(onesrow, onesrow_f)
        negrow_f = constp.tile([1, XC], f32)
        nc.vector.memset(negrow_f, -1.0e9)
        negrow = constp.tile([1, XC], bf16)
        nc.vector.tensor_copy(negrow, negrow_f)
        # m0n[k,q] = 0 if k<=q else -1e9 ; m8n[k,q] = 0 if k>=q else -1e9
        m0n = constp.tile([128, 128], bf16)
        nc.scalar.dma_start(m0n, m0_d[:, :])
        m8n = constp.tile([128, 128], bf16)
        nc.scalar.dma_start(m8n, m8_d[:, :])
        bq_sb = constp.tile([HD, QH], f32)
        nc.scalar.dma_start(bq_sb, bq_d[:, :])
        bk_sb = constp.tile([HD, 1], f32)
        nc.scalar.dma_start(bk_sb, bk_d[:, :])
        bv_sb = constp.tile([1, HD], bf16)
        nc.scalar.dma_start(bv_sb, bv_d[:, :])

        pers = ctx.enter_context(tc.tile_pool(name=f"persist{rep}", bufs=1))
        k_sb = pers.tile([128, S], bf16)
        v_sb = pers.tile([128, S], bf16)

        wp = ctx.enter_context(tc.tile_pool(name=f"weights{rep}", bufs=1))
        wk_sb = wp.tile([128, NE * HD], bf16)
        nc.sync.dma_start(
            wk_sb.rearrange("p (a m) -> p a m", a=NE),
            wkT.rearrange("(a p) m -> a p m", p=128).transpose([1, 0, 2]),
        )
        cos_sb = wp.tile([HD, S], f32)
        nc.scalar.dma_start(cos_sb, cos_d[:, :])
        sin_sb = wp.tile([HD, S], f32)
        nc.scalar.dma_start(sin_sb, sin_d[:, :])
        wq_sb = wp.tile([128, NE * QH * HD], bf16)
        wqT_v = wqT.rearrange("(a p) m -> a p m", p=128)
        wv_sb = wp.tile([128, NE * HD], bf16)
        wo_sb = wp.tile([128, QH * EMB], bf16)

        # psum pools: 2x super(2 banks) + 2x mm + av + dn = 8 banks
        mmp = ctx.enter_context(tc.tile_pool(name=f"mmp{rep}", bufs=2, space="PSUM"))
        sup = ctx.enter_context(tc.tile_pool(name=f"sup{rep}", bufs=2, space="PSUM"))
        avp = ctx.enter_context(tc.tile_pool(name=f"avp{rep}", bufs=1, space="PSUM"))
        dnp = ctx.enter_context(tc.tile_pool(name=f"dnp{rep}", bufs=1, space="PSUM"))

        xp = ctx.enter_context(tc.tile_pool(name=f"xin{rep}", bufs=2))
        tpool = ctx.enter_context(tc.tile_pool(name=f"ptmp{rep}", bufs=3))
        qp = ctx.enter_context(tc.tile_pool(name=f"qbuf{rep}", bufs=2))
        ap = ctx.enter_context(tc.tile_pool(name=f"abuf{rep}", bufs=2))
        ep = ctx.enter_context(tc.tile_pool(name=f"expp{rep}", bufs=14))
        nr = ctx.enter_context(tc.tile_pool(name=f"nrm{rep}", bufs=2))
        outp = ctx.enter_context(tc.tile_pool(name=f"outp{rep}", bufs=3))

        def proj(xt, w_sb, wstride, col0, bias_ap, out_dt):
            ps = mmp.tile([128, XC], f32, tag="mm")
            for e in range(NE):
                nc.tensor.matmul(
                    ps,
                    w_sb[:, e * wstride + col0 : e * wstride + col0 + HD],
                    xt[:, e * XC : (e + 1) * XC],
                    start=(e == 0),
                    stop=(e == NE - 1),
                )
            raw = tpool.tile([128, XC], out_dt, tag="praw")
            nc.scalar.activation(raw, ps, AF.Identity, bias=bias_ap)
            return raw

        def rope(raw, sl, dst):
            t1 = tpool.tile([128, XC], f32, tag="t1")
            t2 = tpool.tile([128, XC], f32, tag="t2")
            nc.scalar.dma_start(t2[0:64, :], raw[64:128, :])
            nc.scalar.dma_start(t2[64:128, :], raw[0:64, :])
            nc.vector.tensor_mul(t1, raw, cos_sb[:, sl])
            nc.vector.tensor_mul(t2, t2, sin_sb[:, sl])
            nc.vector.tensor_add(dst, t1, t2)

        for c in range(NCH):
            sl = slice(c * XC, (c + 1) * XC)
            xt = xp.tile([128, NE * XC], bf16, tag="xt")
            xT_v = xT[:, sl].rearrange("(a p) n -> a p n", p=128)
            half = NE // 2
            for hv in range(2):
                nc.sync.dma_start(
                    xt[:, hv * half * XC : (hv + 1) * half * XC].rearrange(
                        "p (a n) -> p a n", a=half
                    ),
                    xT_v[hv * half : (hv + 1) * half].transpose([1, 0, 2]),
                )
            if c == 0:
                for e in range(NE):
                    nc.sync.dma_start(
                        wq_sb[:, e * QH * HD : (e + 1) * QH * HD], wqT_v[e]
                    )
                nc.sync.dma_start(
                    wv_sb.rearrange("p (a m) -> p a m", a=NE),
                    wvT.rearrange("(a p) m -> a p m", p=128).transpose([1, 0, 2]),
                )
                nc.sync.dma_start(
                    wo_sb.rearrange("p (a m) -> p a m", a=QH),
                    woT.rearrange("(a p) m -> a p m", p=128).transpose([1, 0, 2]),
                )

            # ---- K projection + rope ----
            kraw = proj(xt, wk_sb, HD, 0, bk_sb[:, 0:1], f32)
            rope(kraw, sl, k_sb[:, sl])

            # ---- V direct in [seq, hd] layout ----
            for j in range(XC // 128):
                vps = mmp.tile([128, 128], f32, tag="mm")
                for e in range(NE):
                    nc.tensor.matmul(
                        vps,
                        xt[:, e * XC + j * 128 : e * XC + (j + 1) * 128],
                        wv_sb[:, e * HD : (e + 1) * HD],
                        start=(e == 0),
                        stop=False,
                    )
                nc.tensor.matmul(
                    vps, onesrow[:, 0:128], bv_sb, start=False, stop=True
                )
                t0 = (c * XC) // 128 + j
                nc.vector.tensor_copy(v_sb[:, t0 * 128 : (t0 + 1) * 128], vps)

            # ---- Q projections + rope; layout [sub(2) x head(4) x QC] ----
            q_sb = qp.tile([128, 2 * QH * QC], bf16, tag="q")
            q_v = q_sb.rearrange("p (s h q) -> p s h q", s=2, h=QH)
            for h in range(QH):
                qraw = proj(xt, wq_sb, QH * HD, h * HD, bq_sb[:, h : h + 1], f32)
                rope(qraw, sl, q_v[:, :, h, :])

            attn_sb = ap.tile([128, QH * XC], bf16, tag="attn")
            attn_v = attn_sb.rearrange("p (h s q) -> p h s q", h=QH, s=2)

            # ---- attention: sub-chunks x head-pairs ----
            for s in range(2):
                cq = 2 * c + s
                kt_lo = max(0, 2 * cq - 8)
                kts = list(range(kt_lo, 2 * cq + 2))
                n = len(kts)
                for p in range(2):
                    h0 = 2 * p
                    q_pair = q_sb[
                        :, s * QH * QC + h0 * QC : s * QH * QC + (h0 + 2) * QC
                    ]
                    q3 = q_pair.rearrange("p (h q) -> p h q", h=2)
                    # pair kts into [128,1024] super-tiles; one exp each
                    etiles = []  # list of (et_supertile, n_halves)
                    ethalves = []

                    def emit_mask(psv, d0, last):
                        # psv: [128, 512] view (one kt) as (h, q) blocks
                        s3 = psv.rearrange("p (h q) -> p h q", h=2)
                        if d0 == -1:
                            nc.tensor.matmul(
                                s3[:, 0, 0:128], onesrow[:, 0:128],
                                negrow[:, 0:128], start=False, stop=False,
                            )
                            nc.tensor.matmul(
                                s3[:, 1, 0:128], onesrow[:, 0:128],
                                negrow[:, 0:128], start=False, stop=False,
                            )
                            nc.tensor.matmul(
                                s3[:, 0, 128:256], ident, m0n,
                                start=False, stop=False,
                            )
                            nc.tensor.matmul(
                                s3[:, 1, 128:256], ident, m0n,
                                start=False, stop=last,
                            )
                        elif d0 == 0:
                            nc.tensor.matmul(
                                s3[:, 0, 0:128], ident, m0n,
                                start=False, stop=False,
                            )
                            nc.tensor.matmul(
                                s3[:, 1, 0:128], ident, m0n,
                                start=False, stop=last,
                            )
                        elif d0 == 7:
                            nc.tensor.matmul(
                                s3[:, 0, 128:256], ident, m8n,
                                start=False, stop=False,
                            )
                            nc.tensor.matmul(
                                s3[:, 1, 128:256], ident, m8n,
                                start=False, stop=last,
                            )
                        elif d0 == 8:
                            nc.tensor.matmul(
                                s3[:, 0, 0:128], ident, m8n,
                                start=False, stop=False,
                            )
                            nc.tensor.matmul(
                                s3[:, 1, 0:128], ident, m8n,
                                start=False, stop=False,
                            )
                            nc.tensor.matmul(
                                s3[:, 0, 128:256], onesrow[:, 0:128],
                                negrow[:, 0:128], start=False, stop=False,
                            )
                            nc.tensor.matmul(
                                s3[:, 1, 128:256], onesrow[:, 0:128],
                                negrow[:, 0:128], start=False, stop=last,
                            )

                    for gi in range(0, n, 2):
                        pair = kts[gi : gi + 2]
                        ssp = sup.tile([128, 4 * QC], f32, tag="sup")
                        for pi, kt in enumerate(pair):
                            d0 = 2 * cq - kt
                            needs_mask = d0 in (-1, 0, 7, 8)
                            psv = ssp[:, pi * 512 : (pi + 1) * 512]
                            nc.tensor.matmul(
                                psv,
                                k_sb[:, kt * 128 : (kt + 1) * 128],
                                q_pair,
                                start=True,
                                stop=not needs_mask,
                            )
                            if needs_mask:
                                emit_mask(psv, d0, True)
                        et2 = ep.tile([128, 4 * QC], bf16, tag="et")
                        if len(pair) == 2:
                            nc.scalar.activation(et2, ssp, AF.Exp, scale=SCALE)
                            ethalves.append(et2[:, 0:512])
                            ethalves.append(et2[:, 512:1024])
                        else:
                            nc.scalar.activation(
                                et2[:, 0:512], ssp[:, 0:512], AF.Exp, scale=SCALE
                            )
                            ethalves.append(et2[:, 0:512])
                    ets = ethalves
                    # denominator on PE
                    dn = dnp.tile([1, 2 * QC], f32, tag="dn")
                    for i, et in enumerate(ets):
                        nc.tensor.matmul(
                            dn, ones_bf, et, start=(i == 0), stop=(i == n - 1)
                        )
                    rec_row = nr.tile([1, 2 * QC], f32, tag="rr")
                    nc.vector.reciprocal_approx_fast(rec_row, dn)
                    rec = nr.tile([128, 2 * QC], f32, tag="rec")
                    nc.gpsimd.partition_broadcast(rec, rec_row[0:1, :])
                    av = avp.tile([128, 2 * QC], f32, tag="av")
                    for i, et in enumerate(ets):
                        nc.tensor.matmul(
                            av,
                            v_sb[:, kts[i] * 128 : (kts[i] + 1) * 128],
                            et,
                            start=(i == 0),
                            stop=(i == n - 1),
                        )
                    nc.vector.tensor_mul(
                        attn_v[:, h0 : h0 + 2, s, :],
                        av.rearrange("p (h q) -> p h q", h=2),
                        rec.rearrange("p (h q) -> p h q", h=2),
                    )

            # ---- output projection for this chunk ----
            OC = 512
            for qtl in range(XC // 128):
                qt = (c * XC) // 128 + qtl
                ot = outp.tile([128, EMB], bf16, tag="ot")
                for ep2 in range(2):  # ec pairs
                    ops = sup.tile([128, 2 * OC], f32, tag="sup")
                    for eci in range(2):
                        ec = ep2 * 2 + eci
                        for hh in range(QH):
                            nc.tensor.matmul(
                                ops[:, eci * OC : (eci + 1) * OC],
                                attn_sb[
                                    :,
                                    hh * XC + qtl * 128 : hh * XC + (qtl + 1) * 128,
                                ],
                                wo_sb[:, hh * EMB + ec * OC : hh * EMB + (ec + 1) * OC],
                                start=(hh == 0),
                                stop=(hh == QH - 1),
                            )
                    nc.vector.tensor_copy(
                        ot[:, ep2 * 2 * OC : (ep2 + 1) * 2 * OC], ops
                    )
                nc.gpsimd.dma_start(out_d[qt * 128 : (qt + 1) * 128, :], ot)


def _build_nc(n_reps=1):
    import concourse.mybir as mybir
    import concourse.tile as tile
    from concourse import bacc

    f32 = mybir.dt.float32
    bf16 = mybir.dt.bfloat16

    nc = bacc.Bacc("TRN2", target_bir_lowering=False, debug=False)

    xT = nc.dram_tensor("xT", [EMB, S], bf16, kind="ExternalInput")
    wqT = nc.dram_tensor("wqT", [EMB, QH * HD], bf16, kind="ExternalInput")
    wkT = nc.dram_tensor("wkT", [EMB, HD], bf16, kind="ExternalInput")
    wvT = nc.dram_tensor("wvT", [EMB, HD], bf16, kind="ExternalInput")
    woT = nc.dram_tensor("woT", [QH * HD, EMB], bf16, kind="ExternalInput")
    bq_d = nc.dram_tensor("bq", [HD, QH], f32, kind="ExternalInput")
    bk_d = nc.dram_tensor("bk", [HD, 1], f32, kind="ExternalInput")
    bv_d = nc.dram_tensor("bv", [1, HD], bf16, kind="ExternalInput")
    cos_d = nc.dram_tensor("cosT", [HD, S], f32, kind="ExternalInput")
    sin_d = nc.dram_tensor("sinT", [HD, S], f32, kind="ExternalInput")
    m0_d = nc.dram_tensor("mask0", [128, 128], bf16, kind="ExternalInput")
    m8_d = nc.dram_tensor("mask8", [128, 128], bf16, kind="ExternalInput")
    out_d = nc.dram_tensor("out", [S, EMB], bf16, kind="ExternalOutput")

    d = (xT, wqT, wkT, wvT, woT, bq_d, bk_d, bv_d, cos_d, sin_d, m0_d, m8_d, out_d)

    with tile.TileContext(nc) as tc:
        for rep in range(n_reps):
            if rep > 0:
                tc.strict_bb_all_engine_barrier()
            _emit_body(nc, tc, d, rep)

    nc.compile()
    return nc


def _get_nc(n_reps=1):
    key = ("nc", n_reps)
    if key not in _NC_CACHE:
        _NC_CACHE[key] = _build_nc(n_reps)
    return _NC_CACHE[key]


def _get_runner(n_reps=1):
    key = ("runner", n_reps)
    if key in _NC_CACHE:
        return _NC_CACHE[key]

    import jax
    from jax.experimental.shard_map import shard_map
    from jax.sharding import Mesh, NamedSharding, PartitionSpec

    import concourse.mybir as mybir
    from concourse import bass2jax

    nc = _get_nc(n_reps)
    bass2jax.install_neuronx_cc_hook()

    partition_name = nc.partition_id_tensor.name if nc.partition_id_tensor else None
    in_names, out_names, out_avals, zero_outs = [], [], [], []
    for alloc in nc.m.functions[0].allocations:
        if not isinstance(alloc, mybir.MemoryLocationSet):
            continue
        name = alloc.memorylocations[0].name
        if alloc.kind == "ExternalInput":
            if name != partition_name:
                in_names.append(name)
        elif alloc.kind == "ExternalOutput":
            shape = tuple(alloc.tensor_shape)
            dtype = mybir.dt.np(alloc.dtype)
            out_avals.append(jax.core.ShapedArray(shape, dtype))
            out_names.append(name)
            zero_outs.append(np.zeros(shape, dtype))
    n_params = len(in_names)
    all_names = in_names + out_names
    if partition_name is not None:
        all_names = all_names + [partition_name]

    def _body(*args):
        operands = list(args)
        if partition_name is not None:
            operands.append(bass2jax.partition_id_tensor())
        outs = bass2jax._bass_exec_p.bind(
            *operands,
            out_avals=tuple(out_avals),
            in_names=tuple(all_names),
            out_names=tuple(out_names),
            lowering_input_output_aliases=(),
            sim_require_finite=True,
            sim_require_nnan=True,
            nc=nc,
        )
        return tuple(outs)

    n_cores = 8
    devices = jax.devices()[:n_cores]
    mesh = Mesh(np.asarray(devices), ("core",))
    spec = PartitionSpec("core")
    sharded = jax.jit(
        shard_map(
            _body,
            mesh=mesh,
            in_specs=(spec,) * (n_params + len(out_names)),
            out_specs=(spec,) * len(out_names),
            check_rep=False,
        ),
        keep_unused=True,
    )
    sharding = NamedSharding(mesh, spec)
    runner = (sharded, in_names, out_names, out_avals, zero_outs, sharding)
    _NC_CACHE[key] = runner
    return runner


def _device_inputs(in_maps, n_reps=1):
    import jax

    sharded, in_names, out_names, out_avals, zero_outs, sharding = _get_runner(n_reps)
    arrs = []
    for name in in_names:
        cat = np.concatenate([np.asarray(m[name]) for m in in_maps], axis=0)
        arrs.append(jax.device_put(cat, sharding))
    for z in zero_outs:
        cat = np.zeros((8 * z.shape[0], *z.shape[1:]), z.dtype)
        arrs.append(jax.device_put(cat, sharding))
    return arrs


def _run_on_device(dev_args, n_reps=1):
    sharded, in_names, out_names, out_avals, zero_outs, sharding = _get_runner(n_reps)
    out_arrs = sharded(*dev_args)
    results = []
    for c in range(8):
        results.append(
            {
                name: np.asarray(out_arrs[i]).reshape(8, *out_avals[i].shape)[c]
                for i, name in enumerate(out_names)
            }
        )
    return results


def _host_prep(x, Wq, bq, Wk, bk, Wv, bv, Wo, bo):
    import ml_dtypes

    bf = ml_dtypes.bfloat16
    pos = np.arange(S, dtype=np.float64)
    inv_freq = 1.0 / (ROPE_THETA ** (np.arange(0, HD, 2, dtype=np.float64) / HD))
    freqs = pos[None, :] * inv_freq[:, None]
    cosT = np.empty((HD, S), np.float32)
    cosT[0:64] = np.cos(freqs)
    cosT[64:128] = np.cos(freqs)
    sinT = np.empty((HD, S), np.float32)
    sinT[0:64] = -np.sin(freqs)
    sinT[64:128] = np.sin(freqs)

    ii = np.arange(128)
    mask0 = np.where(ii[:, None] <= ii[None, :], 0.0, -1.0e9).astype(bf)
    mask8 = np.where(ii[:, None] >= ii[None, :], 0.0, -1.0e9).astype(bf)

    in_maps = []
    for core in range(8):
        b, g = core // NKV, core % NKV
        qs = slice(g * QH * HD, (g + 1) * QH * HD)
        ks = slice(g * HD, (g + 1) * HD)
        in_maps.append(
            {
                "xT": np.ascontiguousarray(x[b].T).astype(bf),
                "wqT": np.ascontiguousarray(Wq[qs].T).astype(bf),
                "wkT": np.ascontiguousarray(Wk[ks].T).astype(bf),
                "wvT": np.ascontiguousarray(Wv[ks].T).astype(bf),
                "woT": np.ascontiguousarray(Wo[:, qs].T).astype(bf),
                "bq": np.ascontiguousarray(bq[qs].reshape(QH, HD).T),
                "bk": np.ascontiguousarray(bk[ks].reshape(1, HD).T),
                "bv": bv[ks].reshape(1, HD).astype(bf),
                "cosT": cosT,
                "sinT": sinT,
                "mask0": mask0,
                "mask8": mask8,
            }
        )
    return in_maps


def _bench_reps(inputs, n_reps, iters):
    import time

    import jax

    in_maps = _host_prep(
        np.asarray(inputs["x"], np.float32),
        np.asarray(inputs["Wq"], np.float32),
        np.asarray(inputs["bq"], np.float32),
        np.asarray(inputs["Wk"], np.float32),
        np.asarray(inputs["bk"], np.float32),
        np.asarray(inputs["Wv"], np.float32),
        np.asarray(inputs["bv"], np.float32),
        np.asarray(inputs["Wo"], np.float32),
        np.asarray(inputs["bo"], np.float32),
    )
    dev_args = _device_inputs(in_maps, n_reps)
    sharded = _get_runner(n_reps)[0]
    jax.block_until_ready(sharded(*dev_args))
    for _ in range(5):
        outs = sharded(*dev_args)
    jax.block_until_ready(outs)
    t0 = time.perf_counter()
    outs = None
    for _ in range(iters):
        outs = sharded(*dev_args)
    jax.block_until_ready(outs)
    t1 = time.perf_counter()
    return (t1 - t0) / iters * 1e9


def bench_slope_ns(inputs, k1=8, k2=24, iters=60):
    t1 = _bench_reps(inputs, k1, iters)
    t2 = _bench_reps(inputs, k2, iters)
    return (t2 - t1) / (k2 - k1)


def kernel(**inputs):
    x = np.asarray(inputs["x"], np.float32)
    bo = np.asarray(inputs["bo"], np.float32)
    in_maps = _host_prep(
        x,
        np.asarray(inputs["Wq"], np.float32),
        np.asarray(inputs["bq"], np.float32),
        np.asarray(inputs["Wk"], np.float32),
        np.asarray(inputs["bk"], np.float32),
        np.asarray(inputs["Wv"], np.float32),
        np.asarray(inputs["bv"], np.float32),
        np.asarray(inputs["Wo"], np.float32),
        bo,
    )
    results = _run_on_device(_device_inputs(in_maps))

    out = np.empty((2, S, EMB), np.float32)
    for b in range(2):
        acc = results[b * NKV]["out"].astype(np.float32)
        for g in range(1, NKV):
            acc += results[b * NKV + g]["out"].astype(np.float32)
        out[b] = acc + bo[None, :]
    return out


def bench_ns(inputs, iters=60):
    """Per-execution device time (ns). Measured as d(wall)/d(reps) between
    two NEFFs containing 8 and 24 barrier-separated executions of the
    kernel body; the slope cancels host/dispatch overhead, leaving the
    hardware execution time of one kernel instance."""
    return bench_slope_ns(inputs, k1=8, k2=24, iters=iters)
